# revision 8
# baseline (speedup 1.0000x reference)
"""AlphaKGNNStage distributed Trainium2 kernel (8 NeuronCores).

Math: for each layer t:
    x = l2norm(x + relu(sum_k softmax(alpha)[k] * GCNConv_t(x, A_k)))
Because the hop masks are disjoint and softmax(alpha) sums to 1, the inner
k-sum collapses to a single weighted scatter:
    agg[n] = sum_{e: dst_e=n} w_e * xw[src_e] + selfcoef[n] * xw[n] + b[t]
    w_e = a[k_e] * rsqrt(deg_{k_e}[src_e]) * rsqrt(deg_{k_e}[dst_e])
    selfcoef[n] = sum_k a[k] / deg_k[n]
with deg_k[n] = (#edges of hop k into n) + 1. All w/deg/selfcoef are
graph-static and precomputed on host.

Distribution: nodes are sharded 8 x NPB (dst-owner shards edges). Per layer,
each core computes its xw shard (PE), AllGathers a bf16 xw table, gathers its
edges' source rows via indirect DMA (dst-block-sorted, 128-edge chunks),
applies the scatter as one-hot-times-weight matmuls (host-baked S tiles,
streamed) accumulating in PSUM per 128-node dst block, then applies
self-term + bias + relu + residual + l2-normalize.

SPMD: chunk schedule is shared across cores (per-block chunk count = max
over cores), with zero-weight padding edges.
"""
import math
import os

import numpy as np
import ml_dtypes

import concourse.bass as bass
import concourse.bacc as bacc
import concourse.tile as tile
from concourse import mybir
from concourse.bass_utils import run_bass_kernel_spmd
from concourse.masks import make_identity

NCORES = 8
D = 128
P = 128
SLAB = 32  # chunks per indirect-DMA slab (4096 edges)

LAST_RESULT = {}  # exec_time_ns etc. stashed here for test harness


def _softmax(v):
    v = v.astype(np.float64)
    m = np.exp(v - v.max())
    return (m / m.sum()).astype(np.float32)


def _preprocess(x, edge_index, edge_attr, W, b, alpha):
    """Host-side graph preprocessing. Returns per-core inputs + schedule."""
    N = x.shape[0]
    L = W.shape[0]
    K = alpha.shape[0]
    NPB = int(math.ceil(N / (NCORES * P))) * P  # nodes per core (padded)
    NPAD = NCORES * NPB
    NB = NPB // P  # dst blocks per core

    src = np.asarray(edge_index[0], dtype=np.int64)
    dst = np.asarray(edge_index[1], dtype=np.int64)
    ek = np.asarray(edge_attr, dtype=np.int64)
    a = _softmax(np.asarray(alpha))

    deg = np.ones((K, N), dtype=np.float64)
    for kk in range(K):
        deg[kk] += np.bincount(dst[ek == kk], minlength=N)
    dinv = 1.0 / np.sqrt(deg)
    w_e = (a[ek] * dinv[ek, src] * dinv[ek, dst]).astype(np.float32)
    selfcoef = np.zeros(NPAD, dtype=np.float32)
    selfcoef[:N] = (a[:, None] / deg).sum(axis=0).astype(np.float32)

    # per-core edge partition by dst owner; count edges per (core, block)
    core_of = dst // NPB
    blk_of = (dst % NPB) >> 7
    cnt = np.zeros((NCORES, NB), dtype=np.int64)
    np.add.at(cnt, (core_of, blk_of), 1)
    nchk = np.maximum(1, (np.ceil(cnt / P)).astype(np.int64).max(axis=0))  # [NB]
    chunk_base = np.zeros(NB + 1, dtype=np.int64)
    chunk_base[1:] = np.cumsum(nchk)
    TC = int(chunk_base[-1])  # total chunks per layer (same all cores)
    chunk_block = np.repeat(np.arange(NB), nchk)  # [TC]

    gidx_all = []
    smat_all = []
    for c in range(NCORES):
        sel = np.nonzero(core_of == c)[0]
        dl = dst[sel] - c * NPB
        blk = dl >> 7
        order = np.argsort(blk, kind="stable")
        blk_s = blk[order]
        src_s = src[sel][order]
        dl_s = dl[order]
        w_s = w_e[sel][order]
        starts = np.searchsorted(blk_s, np.arange(NB))
        posin = np.arange(len(sel)) - starts[blk_s]
        chunk = chunk_base[blk_s] + (posin >> 7)
        part = posin & 127
        gidx = np.zeros((P, TC), dtype=np.int32)
        gidx[part, chunk] = src_s
        smat = np.zeros((P, TC * P), dtype=np.float32)
        smat[part, chunk * P + (dl_s & 127)] = w_s
        gidx_all.append(gidx)
        smat_all.append(smat.astype(ml_dtypes.bfloat16))

    xs = []
    sc = []
    for c in range(NCORES):
        xc = np.zeros((NPB, D), dtype=np.float32)
        lo, hi = c * NPB, min((c + 1) * NPB, N)
        xc[: hi - lo] = np.asarray(x[lo:hi], dtype=np.float32)
        xs.append(xc)
        scc = selfcoef[c * NPB:(c + 1) * NPB].reshape(NB, P).T.copy()  # [P, NB]
        sc.append(scc)

    meta = dict(N=N, L=L, NPB=NPB, NPAD=NPAD, NB=NB, TC=TC,
                chunk_block=chunk_block, nchk=nchk,
                has_bias=bool(np.any(np.asarray(b))))
    W32 = np.asarray(W, dtype=np.float32)
    b32 = np.asarray(b, dtype=np.float32)
    return meta, xs, gidx_all, smat_all, sc, W32, b32


def _build(meta):
    L, NPB, NPAD, NB, TC = meta["L"], meta["NPB"], meta["NPAD"], meta["NB"], meta["TC"]
    chunk_block = meta["chunk_block"]
    has_bias = meta["has_bias"]
    AF = mybir.ActivationFunctionType
    OP = mybir.AluOpType
    f32 = mybir.dt.float32
    bf16 = mybir.dt.bfloat16

    nc = bacc.Bacc("TRN2", target_bir_lowering=False, debug=False,
                   num_devices=NCORES)
    x_in = nc.declare_dram_parameter("x", [NPB, D], f32, isOutput=False)
    gidx_in = nc.declare_dram_parameter("gidx", [P, TC], mybir.dt.int32, isOutput=False)
    smat_in = nc.declare_dram_parameter("smat", [P, TC * P], bf16, isOutput=False)
    selfc_in = nc.declare_dram_parameter("selfc", [P, NB], f32, isOutput=False)
    w_in = nc.declare_dram_parameter("W", [L, D, D], f32, isOutput=False)
    b_in = nc.declare_dram_parameter("b", [L, D], f32, isOutput=False)
    out_p = nc.declare_dram_parameter("out", [NPB, D], f32, isOutput=True)

    with tile.TileContext(nc) as tc:
        with tc.tile_pool(name="dram", bufs=1, space="DRAM") as dram, \
             tc.tile_pool(name="singles", bufs=1) as sing, \
             tc.tile_pool(name="xtp", bufs=3) as xtp, \
             tc.tile_pool(name="msgp", bufs=8) as msgp, \
             tc.tile_pool(name="spool", bufs=4) as spool, \
             tc.tile_pool(name="scr", bufs=4) as scr, \
             tc.tile_pool(name="psA", bufs=2, space="PSUM") as psA, \
             tc.tile_pool(name="psB", bufs=2, space="PSUM") as psB, \
             tc.tile_pool(name="psS", bufs=3, space="PSUM") as psS:

            bounces = [dram.tile([NPB, D], bf16, name=f"bounce{t}") for t in range(L)]
            tables = [dram.tile([NPAD, D], bf16, addr_space="Shared", name=f"table{t}")
                      for t in range(L)]

            # persistent SBUF state
            x_sb = sing.tile([P, NB, D], f32)
            nc.sync.dma_start(out=x_sb[:], in_=x_in[:].rearrange("(b p) d -> p b d", p=P))
            gidx_sb = sing.tile([P, TC], mybir.dt.int32)
            nc.sync.dma_start(out=gidx_sb[:], in_=gidx_in[:])
            selfc_sb = sing.tile([P, NB], f32)
            nc.sync.dma_start(out=selfc_sb[:], in_=selfc_in[:])
            xw_sb = sing.tile([P, NB, D], bf16)
            ident = sing.tile([P, P], f32)
            make_identity(nc, ident[:])
            ones_bf = sing.tile([1, P], bf16)
            nc.vector.memset(ones_bf, 1.0)
            w_bf = []
            b_bf = []
            for t in range(L):
                wt = sing.tile([P, D], f32, name=f"w32_{t}")
                nc.sync.dma_start(out=wt[:], in_=w_in[t])
                wb = sing.tile([P, D], bf16, name=f"wbf_{t}")
                nc.vector.tensor_copy(out=wb[:], in_=wt[:])
                w_bf.append(wb)
                if has_bias:
                    bt = sing.tile([1, D], f32, name=f"b32_{t}")
                    nc.sync.dma_start(out=bt[:], in_=b_in[t:t + 1, :])
                    bb = sing.tile([1, D], bf16, name=f"bbf_{t}")
                    nc.vector.tensor_copy(out=bb[:], in_=bt[:])
                    b_bf.append(bb)
            ss = sing.tile([P, NB], f32)       # sum of squares per node
            rn = sing.tile([P, NB], f32)       # 1/norm per node
            eps = sing.tile([P, 1], f32)
            nc.vector.memset(eps, 1e-24)

            for t in range(L):
                # ---- phase X: xw^ = (x @ W[t]) in bf16, block by block ----
                for nb in range(NB):
                    xt_ps = psA.tile([P, P], f32, name="xt_ps")
                    nc.tensor.transpose(xt_ps[:], x_sb[:, nb, :], ident[:])
                    xt_bf = xtp.tile([P, P], bf16, name="xt_bf")
                    nc.scalar.activation(out=xt_bf[:], in_=xt_ps[:], func=AF.Copy)
                    xw_ps = psB.tile([P, D], f32, name="xw_ps")
                    nc.tensor.matmul(out=xw_ps[:], lhsT=xt_bf[:], rhs=w_bf[t][:],
                                     start=True, stop=True)
                    nc.scalar.activation(out=xw_sb[:, nb, :], in_=xw_ps[:], func=AF.Copy)
                nc.sync.dma_start(out=bounces[t][:].rearrange("(b p) d -> p b d", p=P),
                                  in_=xw_sb[:])
                nc.gpsimd.collective_compute(
                    "AllGather", OP.bypass,
                    replica_groups=[list(range(NCORES))],
                    ins=[bounces[t].opt()], outs=[tables[t].opt()])

                # ---- phase E: gather + scatter-matmul per chunk ----
                cur_ps = None
                for c0 in range(0, TC, SLAB):
                    cols = min(SLAB, TC - c0)
                    ssb = spool.tile([P, SLAB * P], bf16, name="ssb")
                    nc.sync.dma_start(out=ssb[:, :cols * P],
                                      in_=smat_in[:, c0 * P:(c0 + cols) * P])
                    for j in range(cols):
                        ch = c0 + j
                        # one indirect gather per 128-edge chunk: the walrus
                        # dynamic-DMA path only honors one index per partition
                        msg = msgp.tile([P, D], bf16, name="msg")
                        nc.gpsimd.indirect_dma_start(
                            out=msg[:], out_offset=None,
                            in_=tables[t][:],
                            in_offset=bass.IndirectOffsetOnAxis(
                                ap=gidx_sb[:, ch:ch + 1], axis=0))
                        blk = int(chunk_block[ch])
                        first = ch == 0 or int(chunk_block[ch - 1]) != blk
                        last = ch == TC - 1 or int(chunk_block[ch + 1]) != blk
                        if first:
                            cur_ps = psS.tile([P, D], f32, name="agg_ps")
                        nc.tensor.matmul(out=cur_ps[:],
                                         lhsT=ssb[:, j * P:(j + 1) * P],
                                         rhs=msg[:],
                                         start=first,
                                         stop=last and not has_bias)
                        if last:
                            if has_bias:
                                nc.tensor.matmul(out=cur_ps[:], lhsT=ones_bf[:],
                                                 rhs=b_bf[t][:], start=False, stop=True)
                            # ---- post: self-term + relu + residual + sumsq ----
                            st = scr.tile([P, D], f32, name="st")
                            nc.vector.tensor_tensor(
                                out=st[:], in0=xw_sb[:, blk, :],
                                in1=selfc_sb[:, blk:blk + 1].to_broadcast([P, D]),
                                op=OP.mult)
                            s2 = scr.tile([P, D], f32, name="s2")
                            nc.vector.tensor_tensor(out=s2[:], in0=cur_ps[:],
                                                    in1=st[:], op=OP.add)
                            nc.scalar.activation(out=s2[:], in_=s2[:], func=AF.Relu)
                            nc.vector.tensor_tensor(out=x_sb[:, blk, :], in0=s2[:],
                                                    in1=x_sb[:, blk, :], op=OP.add)
                            sq = scr.tile([P, D], f32, name="sq")
                            nc.scalar.activation(out=sq[:], in_=x_sb[:, blk, :],
                                                 func=AF.Square,
                                                 accum_out=ss[:, blk:blk + 1])
                # ---- normalize: x /= sqrt(ss + eps) ----
                nc.scalar.activation(out=rn[:], in_=ss[:], func=AF.Sqrt, bias=eps[:])
                nc.vector.reciprocal(out=rn[:], in_=rn[:])
                rn_ap = rn[:]
                rn_b = bass.AP(tensor=rn_ap.tensor, offset=rn_ap.offset,
                               ap=[rn_ap.ap[0], rn_ap.ap[1], [0, D]])
                nc.vector.tensor_tensor(out=x_sb[:], in0=x_sb[:], in1=rn_b, op=OP.mult)

            nc.sync.dma_start(out=out_p[:].rearrange("(b p) d -> p b d", p=P),
                              in_=x_sb[:])
    nc.compile()
    return nc


def kernel(x, edge_index, edge_attr, W, b, alpha):
    meta, xs, gidx_all, smat_all, sc, W32, b32 = _preprocess(
        x, edge_index, edge_attr, W, b, alpha)
    nc = _build(meta)
    in_maps = [
        {"x": xs[c], "gidx": gidx_all[c], "smat": smat_all[c], "selfc": sc[c],
         "W": W32, "b": b32}
        for c in range(NCORES)
    ]
    trace = bool(int(os.environ.get("BENCH_TRACE", "0")))
    if trace:
        _install_ntff_hook()
    res = run_bass_kernel_spmd(nc, in_maps, core_ids=list(range(NCORES)),
                               trace=trace)
    LAST_RESULT["exec_time_ns"] = res.exec_time_ns
    LAST_RESULT["res"] = res
    LAST_RESULT["scope_times"] = res.per_core_scope_times
    N, NPB = meta["N"], meta["NPB"]
    out = np.empty((N, D), dtype=np.float32)
    for c in range(NCORES):
        lo, hi = c * NPB, min((c + 1) * NPB, N)
        if hi > lo:
            out[lo:hi] = res.results[c]["out"][: hi - lo]
    return out


def _install_ntff_hook():
    """Shim antenv.axon_hooks so run_bass_kernel_spmd(trace=True) can profile."""
    import sys
    import types
    import antenv
    if "antenv.axon_hooks" in sys.modules:
        return
    mod = types.ModuleType("antenv.axon_hooks")
    mod._hook = None
    mod.set_axon_ntff_profile_hook = lambda h: setattr(mod, "_hook", h)
    mod.get_axon_ntff_profile_hook = lambda: mod._hook
    sys.modules["antenv.axon_hooks"] = mod
    antenv.axon_hooks = mod
    try:
        from trn_agent_boot.trn_boot import _ntff_profile_via_ctypes
        mod.set_axon_ntff_profile_hook(
            _ntff_profile_via_ctypes("/opt/axon/libaxon_pjrt.so"))
    except Exception:
        pass
